# revision 38
# baseline (speedup 1.0000x reference)
"""Tversky-style mismatch loss on Trainium2 (Bass/Tile), 8-core data-parallel.

Full inputs: net_out/target/max_positiones, each [8, 16, 512, 512] f32.
Sharding: batch dim B=8 across 8 NeuronCores (1 image per core).

Per (image, class) plane only tn = sum(target*net_out), t = sum(target)
and n = sum(net_out) are needed on-device, since fn = t - tn, fp = n - tn.
max_positiones is NEVER loaded: its only use is active = (max t)|(max m)>0,
and the m term is dead unless a target plane is entirely zero -- the host
resolves those (rare-to-nonexistent) planes from the returned t sums by
scanning max_positiones in numpy.  That cuts device HBM traffic from 48 to
32 MiB/core (137.7us -> ~95us; the per-core DMA engine pool sustains ~420
GB/s read, which the trace shows is shared across SWDGE and HWDGE queues,
so 32 MiB ~= 80us of streaming is the hardware floor).

Engine split, all under the stream cadence: DVE scalar_tensor_tensor does
the fused t*n product + per-partition rowsum, ACT (Copy + accum_out) sums
n, PE sums t's columns via a shifted ones-window lhsT so each plane's
column sums accumulate into its own PSUM row.  A final ones-matmul reduces
the [128, NOUT] per-partition accumulator across partitions.  Inputs are
cast f32->bf16 in-flight by the SWDGE DMA (target is 0/1 so exact;
net_out's sums pick up ~1e-6 rel err; overall rel err 5e-7).  Plane 0 goes
f32 over the two HWDGE queues, which start ~2us before the SWDGE ucode
spins up (the engine pool is the shared cap, so starting earlier ends
earlier); the last plane is split into half-planes to shorten the
post-stream drain.  The two tiny outputs leave on different queues.
bufs=5 keeps ~5 tile-sets of prefetch against the machine's intermittent
contended windows (runs land ~95us clean, ~115us when an engine's HBM
path is contended by other tenants).
The tiny [8,16] -> scalar tail runs on host in float64.
"""

import os
import sys

import numpy as np

if "/opt/trn_rl_repo" not in sys.path:
    sys.path.insert(0, "/opt/trn_rl_repo")

B, C, H, W = 8, 16, 512, 512
NCORES = 8
P = 128
FREE = H * W // P  # 2048 f32 per partition per plane
CHUNK = 512  # max fp32 moving free dim per matmul
NCHUNK = FREE // CHUNK  # 4

_CACHE = {}


def _build(C=C, H=H, W=W, debug=False, num_devices=NCORES, m_on="act", bufs=3, cpt=2, m_f32=False, split_first=False, n_on="pe", edge_split=False):
    import concourse.bacc as bacc
    import concourse.mybir as mybir
    import concourse.tile as tile

    P = 128
    FREE = H * W // P
    CHUNK = min(512, FREE)
    NCHUNK = FREE // CHUNK

    f32 = mybir.dt.float32
    bf16 = mybir.dt.bfloat16
    nc = bacc.Bacc(
        "TRN2", target_bir_lowering=False, debug=debug, num_devices=num_devices
    )

    t_in = nc.dram_tensor("t_in", [C, H, W], f32, kind="ExternalInput")
    n_in = nc.dram_tensor("n_in", [C, H, W], f32, kind="ExternalInput")
    m_in = (
        nc.dram_tensor("m_in", [C, H, W], f32, kind="ExternalInput")
        if m_on != "skip"
        else None
    )
    out_tn = nc.dram_tensor("out_tn", [1, 2 * C], f32, kind="ExternalOutput")
    out_tnm = nc.dram_tensor("out_tnm", [C, 3], f32, kind="ExternalOutput")

    # pair of planes g as [128 partitions, 2 x 2048 contiguous f32]
    CPT = cpt if C % cpt == 0 else 1  # planes per DMA tile
    NT = C // CPT
    t_r = t_in.ap().rearrange("(g c) (p a) w -> g p c (a w)", c=CPT, p=P)
    n_r = n_in.ap().rearrange("(g c) (p a) w -> g p c (a w)", c=CPT, p=P)
    m_r = (
        m_in.ap().rearrange("(g c) (p a) w -> g p c (a w)", c=CPT, p=P)
        if m_in is not None
        else None
    )
    # per-plane views for edge (head/tail) chunks
    t_r1 = t_in.ap().rearrange("(g c) (p a) w -> g p c (a w)", c=1, p=P)
    n_r1 = n_in.ap().rearrange("(g c) (p a) w -> g p c (a w)", c=1, p=P)

    if edge_split and m_on == "skip" and CPT == 2 and C >= 6:
        # single planes at both ends: first DMA op has fewer descriptors
        # (stream starts sooner) and the last tiles shrink the drain tail.
        chunks = [(0, 1), (1, 1)]
        chunks += [(c0, 2) for c0 in range(2, C - 2, 2)]
        chunks += [(C - 2, 1), (C - 1, 1)]
    else:
        chunks = [(g * CPT, CPT) for g in range(NT)]

    with tile.TileContext(nc) as tc:
        with (
            tc.tile_pool(name="consts", bufs=1) as consts,
            tc.tile_pool(name="tp", bufs=bufs) as tp,
            tc.tile_pool(name="npool", bufs=bufs) as npool,
            tc.tile_pool(name="mp", bufs=bufs) as mp,
            tc.tile_pool(name="sp", bufs=2) as sp,
            tc.tile_pool(name="spa", bufs=2) as spa,
            tc.tile_pool(name="outp", bufs=1) as outp,
            tc.tile_pool(name="psum", bufs=1, space="PSUM") as psum,
        ):
            ones = consts.tile([P, 1], f32)
            nc.vector.memset(ones[:], 1.0)
            # G[:, C-1] = 1, rest 0.  lhsT window G[:, C-1-c : 2C-1-c] is a
            # [P, C] matrix whose column c is all-ones -> plane c's column
            # sums land in PSUM partition row c, other rows accumulate +0.
            G = consts.tile([P, 2 * C - 1], bf16)
            nc.vector.memset(G[:], 0.0)
            nc.vector.memset(G[:, C - 1 : C], 1.0)
            # per-plane partition-partials: cols [0,C) = t*n, cols [C,2C) = m
            acc = consts.tile([P, 2 * C], f32)
            if m_on in ("pe", "skip"):
                nc.vector.memset(acc[:, C:], 0.0)  # m half unused

            ps_t = psum.tile([C, CHUNK], f32)
            ps_n = psum.tile([C, CHUNK], f32) if n_on == "pe" else None
            ps_m = psum.tile([C, CHUNK], f32, name="ps_m") if m_on == "pe" else None
            ps_tn = psum.tile([1, 2 * C], f32)

            for c0, ncpl in chunks:
                # SWDGE DMAs cast f32 -> bf16 in flight (HWDGE can't cast).
                # target/max_positiones are 0/1-valued so bf16 is exact;
                # net_out's per-plane sums only pick up ~1e-6 rel error.
                tt = tp.tile([P, ncpl * FREE], bf16)
                nt = npool.tile([P, ncpl * FREE], bf16)
                if ncpl == CPT:
                    g = c0 // CPT
                    if split_first and c0 == 0:
                        for q in range(CPT):
                            nc.gpsimd.dma_start(
                                tt[:, q * FREE : (q + 1) * FREE], t_r[g, :, q]
                            )
                            nc.gpsimd.dma_start(
                                nt[:, q * FREE : (q + 1) * FREE], n_r[g, :, q]
                            )
                    else:
                        nc.gpsimd.dma_start(
                            tt[:].rearrange("p (c f) -> p c f", c=CPT), t_r[g]
                        )
                        nc.gpsimd.dma_start(
                            nt[:].rearrange("p (c f) -> p c f", c=CPT), n_r[g]
                        )
                else:
                    assert ncpl == 1
                    nc.gpsimd.dma_start(
                        tt[:].rearrange("p (c f) -> p c f", c=1), t_r1[c0]
                    )
                    nc.gpsimd.dma_start(
                        nt[:].rearrange("p (c f) -> p c f", c=1), n_r1[c0]
                    )
                if m_on == "skip":
                    mt = None
                elif m_f32:
                    # m feeds only the ACT accumulator; load it f32 over
                    # HWDGE to take work off the single SWDGE queue.
                    mt = mp.tile([P, ncpl * FREE], f32, name="mt")
                    nc.sync.dma_start(
                        mt[:].rearrange("p (c f) -> p c f", c=ncpl), m_r[c0 // CPT]
                    )
                else:
                    mt = mp.tile([P, ncpl * FREE], bf16, name="mt")
                    nc.gpsimd.dma_start(
                        mt[:].rearrange("p (c f) -> p c f", c=ncpl), m_r[c0 // CPT]
                    )

                for cc in range(ncpl):
                    c = c0 + cc
                    fsl = slice(cc * FREE, (cc + 1) * FREE)
                    # DVE: fused product + per-partition row sum of t*n.
                    sc = sp.tile([P, FREE], bf16)
                    nc.vector.scalar_tensor_tensor(
                        out=sc[:],
                        in0=tt[:, fsl],
                        scalar=1.0,
                        in1=nt[:, fsl],
                        op0=mybir.AluOpType.mult,
                        op1=mybir.AluOpType.mult,
                        accum_out=acc[:, c : c + 1],
                    )
                    if m_on == "act":
                        # ACT (own SBUF port, otherwise idle): accumulating
                        # sum of m into the accumulator's second half.
                        scm = spa.tile([P, FREE], bf16, name="scm")
                        nc.scalar.activation(
                            scm[:],
                            mt[:, fsl],
                            mybir.ActivationFunctionType.Copy,
                            accum_out=acc[:, C + c : C + c + 1],
                        )
                    if n_on == "act":
                        # ACT (otherwise idle in skip mode): accumulating sum
                        # of net_out into the accumulator's second half, so
                        # PE only carries target's column sums.
                        scn = spa.tile([P, FREE], bf16, name="scn")
                        nc.scalar.activation(
                            scn[:],
                            nt[:, fsl],
                            mybir.ActivationFunctionType.Copy,
                            accum_out=acc[:, C + c : C + c + 1],
                        )

                    w = G[:, C - 1 - c : 2 * C - 1 - c]
                    for k in range(NCHUNK):
                        first = c == 0 and k == 0
                        last = c == C - 1 and k == NCHUNK - 1
                        sl = slice(cc * FREE + k * CHUNK, cc * FREE + (k + 1) * CHUNK)
                        nc.tensor.matmul(
                            ps_t[:, :], w, tt[:, sl], start=first, stop=last
                        )
                        if n_on == "pe":
                            nc.tensor.matmul(
                                ps_n[:, :], w, nt[:, sl], start=first, stop=last
                            )
                        if m_on == "pe":
                            nc.tensor.matmul(
                                ps_m[:, :], w, mt[:, sl], start=first, stop=last
                            )

            # partition-axis total of the tn/m partials: [128, 2C] -> [1, 2C]
            nc.tensor.matmul(ps_tn[:, :], ones[:], acc[:], start=True, stop=True)

            sb_tnm = outp.tile([C, 3], f32)
            nc.vector.reduce_sum(sb_tnm[:, 0:1], ps_t[:], axis=mybir.AxisListType.X)
            if n_on == "pe":
                nc.vector.reduce_sum(
                    sb_tnm[:, 1:2], ps_n[:], axis=mybir.AxisListType.X
                )
            else:
                nc.vector.memset(sb_tnm[:, 1:2], 0.0)  # n sums live in acc
            if m_on == "pe":
                nc.vector.reduce_sum(
                    sb_tnm[:, 2:3], ps_m[:], axis=mybir.AxisListType.X
                )
            else:
                nc.vector.memset(sb_tnm[:, 2:3], 0.0)  # unused in act mode
            sb_tn = outp.tile([1, 2 * C], f32)
            nc.vector.tensor_copy(sb_tn[:], ps_tn[:])

            nc.sync.dma_start(out_tnm.ap(), sb_tnm[:])
            nc.sync.dma_start(out_tn.ap(), sb_tn[:])

    nc.compile()
    return nc


def _build_v5(C=C, H=H, W=W, num_devices=NCORES, bufs=5):
    """v3 engine split (DVE: tn rowsums, ACT: n rowsums, PE: t column sums
    via the shifted ones-window trick) with a half-plane split of the last
    plane to shorten the post-stream drain, and the two output DMAs on
    different queues so they overlap.  m (max_positiones) is never loaded:
    it only matters for planes whose target is entirely zero, which the
    host resolves from the returned t sums."""
    import concourse.bacc as bacc
    import concourse.mybir as mybir
    import concourse.tile as tile

    P = 128
    FREE = H * W // P  # 2048
    HALF = FREE // 2
    CHUNK = min(512, FREE)
    NCHUNK = FREE // CHUNK

    f32 = mybir.dt.float32
    bf16 = mybir.dt.bfloat16
    nc = bacc.Bacc(
        "TRN2", target_bir_lowering=False, debug=False, num_devices=num_devices
    )

    t_in = nc.dram_tensor("t_in", [C, H, W], f32, kind="ExternalInput")
    n_in = nc.dram_tensor("n_in", [C, H, W], f32, kind="ExternalInput")
    # plane 0 goes f32 over the two HWDGE queues, which start ~2us before
    # the SWDGE ucode spins up; the engine pool is the shared cap, so
    # starting earlier ends the stream earlier.
    head_f32 = os.environ.get("K_HEADF32", "1") == "1"
    NOUT = 2 * C + 3  # tn[C] | n[C] | tn15b, n15b, t0
    out_tn = nc.dram_tensor("out_tn", [1, NOUT], f32, kind="ExternalOutput")
    out_tnm = nc.dram_tensor("out_tnm", [C, 1], f32, kind="ExternalOutput")

    # bandwidth probe: stream some never-used f32 planes of m over the
    # sync HWDGE queue to test whether HW queues add DMA bandwidth
    probe_hw = int(os.environ.get("K_PROBE_HW", "0"))
    m_in = (
        nc.dram_tensor("m_in", [C, H, W], f32, kind="ExternalInput")
        if probe_hw
        else None
    )

    t_r2 = t_in.ap().rearrange("(g c) (p a) w -> g p c (a w)", c=2, p=P)
    n_r2 = n_in.ap().rearrange("(g c) (p a) w -> g p c (a w)", c=2, p=P)
    t_rs = t_in.ap().rearrange("c (p a) w -> c p (a w)", p=P)
    n_rs = n_in.ap().rearrange("c (p a) w -> c p (a w)", p=P)
    t_rh = t_in.ap().rearrange("c (h p a) w -> c h p (a w)", p=P, a=2)
    n_rh = n_in.ap().rearrange("c (h p a) w -> c h p (a w)", p=P, a=2)
    m_rs = (
        m_in.ap().rearrange("c (p a) w -> c p (a w)", p=P)
        if m_in is not None
        else None
    )
    # pair-packed view: planes (2g, 2g+1) as [1024, 512]; partition = 8
    # contiguous rows = 16KB contiguous per descriptor (vs 8KB normally)
    m_r16 = (
        m_in.ap().rearrange("(g c) (p2 a) w -> g (c p2) (a w)", c=2, p2=P // 2)
        if m_in is not None
        else None
    )

    chunks = [("single", 0), ("single", 1)]
    chunks += [("pair", c0) for c0 in range(2, C - 2, 2)]
    chunks += [("single", C - 2), ("half", C - 1)]

    with tile.TileContext(nc) as tc:
        with (
            tc.tile_pool(name="consts", bufs=1) as consts,
            tc.tile_pool(name="tp", bufs=bufs) as tp,
            tc.tile_pool(name="npool", bufs=bufs) as npool,
            tc.tile_pool(name="sp", bufs=2) as sp,
            tc.tile_pool(name="spa", bufs=2) as spa,
            tc.tile_pool(name="outp", bufs=1) as outp,
            tc.tile_pool(name="psum", bufs=1, space="PSUM") as psum,
        ):
            ones = consts.tile([P, 1], f32)
            nc.vector.memset(ones[:], 1.0)
            # G[:, C-1] = 1, rest 0: lhsT window G[:, C-1-c : 2C-1-c] is a
            # [P, C] matrix whose column c is all-ones -> plane c's column
            # sums land in PSUM row c, other rows accumulate +0.
            G = consts.tile([P, 2 * C - 1], bf16)
            nc.vector.memset(G[:], 0.0)
            nc.vector.memset(G[:, C - 1 : C], 1.0)
            acc = consts.tile([P, NOUT], f32)
            ps_t = psum.tile([C, CHUNK], f32)
            ps_tn = psum.tile([1, NOUT], f32)

            def reduce_part(tt, nt, fsl, flen, j_tn, j_n, c, mm_sls, stop):
                # DVE: fused t*n product + per-partition rowsum
                sc = sp.tile([P, flen], bf16)
                nc.vector.scalar_tensor_tensor(
                    out=sc[:],
                    in0=tt[:, fsl],
                    scalar=1.0,
                    in1=nt[:, fsl],
                    op0=mybir.AluOpType.mult,
                    op1=mybir.AluOpType.mult,
                    accum_out=acc[:, j_tn : j_tn + 1],
                )
                # ACT: accumulating rowsum of n
                scn = spa.tile([P, flen], bf16, name="scn")
                nc.scalar.activation(
                    scn[:],
                    nt[:, fsl],
                    mybir.ActivationFunctionType.Copy,
                    accum_out=acc[:, j_n : j_n + 1],
                )
                # PE: t's column sums accumulate into ps_t row c
                w = G[:, C - 1 - c : 2 * C - 1 - c]
                for i, sl in enumerate(mm_sls):
                    first = c == (1 if head_f32 else 0) and sl.start == 0
                    nc.tensor.matmul(
                        ps_t[:, :], w, tt[:, sl], start=first,
                        stop=stop and i == len(mm_sls) - 1,
                    )

            probes_issued = 0

            for kind, c0 in chunks:
                if head_f32 and c0 == 0:
                    # plane 0: f32 over both HWDGE queues (earliest start);
                    # its three reductions run engine-side on the f32 tiles
                    # (DVE f32 is 2x slower but overlaps the long stream).
                    tf = tp.tile([P, FREE], f32, name="tf0")
                    nf = npool.tile([P, FREE], f32, name="nf0")
                    nc.sync.dma_start(tf[:], t_rs[0])
                    nc.scalar.dma_start(nf[:], n_rs[0])
                    sc = sp.tile([P, FREE], bf16)
                    nc.vector.scalar_tensor_tensor(
                        out=sc[:],
                        in0=tf[:],
                        scalar=1.0,
                        in1=nf[:],
                        op0=mybir.AluOpType.mult,
                        op1=mybir.AluOpType.mult,
                        accum_out=acc[:, 0:1],
                    )
                    scn = spa.tile([P, FREE], bf16, name="scn")
                    nc.scalar.activation(
                        scn[:],
                        nf[:],
                        mybir.ActivationFunctionType.Copy,
                        accum_out=acc[:, C : C + 1],
                    )
                    sct = spa.tile([P, FREE], bf16, name="sct")
                    nc.scalar.activation(
                        sct[:],
                        tf[:],
                        mybir.ActivationFunctionType.Copy,
                        accum_out=acc[:, 2 * C + 2 : 2 * C + 3],
                    )
                    continue
                if probe_hw and probes_issued < probe_hw and c0 % 2 == 0:
                    # 16KB-descriptor timing probe: pair-packed bf16 load
                    mt = outp.tile([P, 2 * FREE], bf16, name="mt_probe")
                    nc.gpsimd.dma_start(mt[:], m_r16[probes_issued])
                    probes_issued += 1
                # SWDGE DMAs cast f32 -> bf16 in flight (target is 0/1 so
                # exact; net_out's sums pick up ~1e-6 rel error).
                if kind == "pair":
                    tt = tp.tile([P, 2 * FREE], bf16)
                    nt = npool.tile([P, 2 * FREE], bf16)
                    nc.gpsimd.dma_start(
                        tt[:].rearrange("p (c f) -> p c f", c=2), t_r2[c0 // 2]
                    )
                    nc.gpsimd.dma_start(
                        nt[:].rearrange("p (c f) -> p c f", c=2), n_r2[c0 // 2]
                    )
                    for cc in range(2):
                        c = c0 + cc
                        sls = [
                            slice(cc * FREE + k * CHUNK, cc * FREE + (k + 1) * CHUNK)
                            for k in range(NCHUNK)
                        ]
                        reduce_part(
                            tt, nt, slice(cc * FREE, (cc + 1) * FREE), FREE,
                            c, C + c, c, sls, False,
                        )
                elif kind == "single":
                    tt = tp.tile([P, FREE], bf16)
                    nt = npool.tile([P, FREE], bf16)
                    nc.gpsimd.dma_start(tt[:], t_rs[c0])
                    nc.gpsimd.dma_start(nt[:], n_rs[c0])
                    sls = [slice(k * CHUNK, (k + 1) * CHUNK) for k in range(NCHUNK)]
                    reduce_part(
                        tt, nt, slice(0, FREE), FREE, c0, C + c0, c0, sls, False
                    )
                else:  # "half": last plane in two half-plane tiles
                    for h in range(2):
                        tt = tp.tile([P, HALF], bf16)
                        nt = npool.tile([P, HALF], bf16)
                        nc.gpsimd.dma_start(tt[:], t_rh[c0, h])
                        nc.gpsimd.dma_start(nt[:], n_rh[c0, h])
                        sls = [
                            slice(k * CHUNK, (k + 1) * CHUNK)
                            for k in range(HALF // CHUNK)
                        ]
                        if h == 0:
                            reduce_part(
                                tt, nt, slice(0, HALF), HALF,
                                c0, C + c0, c0, sls, False,
                            )
                        else:
                            reduce_part(
                                tt, nt, slice(0, HALF), HALF,
                                2 * C, 2 * C + 1, c0, sls, True,
                            )

            sb_t = outp.tile([C, 1], f32)
            nc.vector.reduce_sum(sb_t[:], ps_t[:], axis=mybir.AxisListType.X)
            nc.tensor.matmul(ps_tn[:, :], ones[:], acc[:], start=True, stop=True)
            sb = outp.tile([1, NOUT], f32)
            nc.vector.tensor_copy(sb[:], ps_tn[:])
            # two small outputs on different queues so they overlap
            nc.scalar.dma_start(out_tnm.ap(), sb_t[:])
            nc.sync.dma_start(out_tn.ap(), sb[:])

    nc.compile()
    return nc


def _build_v4(C=C, H=H, W=W, num_devices=NCORES, bufs=5):
    """skip-m, all-ACT variant: DVE does fused mul+rowsum (tn), ACT does
    accumulating rowsums of t and n.  No PE in the main loop at all (one
    final [1, 3C+3] partition-reduce matmul) -> ~130 fewer Tensor
    instructions, which shrinks the end-of-kernel semaphore drain.
    First DMA op is partition-striped so packets start flowing during
    descriptor generation; the last plane is split into half-planes to
    shorten the post-stream drain (the second half's t-rowsum runs on DVE
    so ACT and DVE drain in parallel)."""
    import concourse.bacc as bacc
    import concourse.mybir as mybir
    import concourse.tile as tile

    P = 128
    FREE = H * W // P  # 2048
    HALF = FREE // 2

    f32 = mybir.dt.float32
    bf16 = mybir.dt.bfloat16
    nc = bacc.Bacc(
        "TRN2", target_bir_lowering=False, debug=False, num_devices=num_devices
    )

    t_in = nc.dram_tensor("t_in", [C, H, W], f32, kind="ExternalInput")
    n_in = nc.dram_tensor("n_in", [C, H, W], f32, kind="ExternalInput")
    NOUT = 3 * C + 3  # tn[C] | n[C] | t[C] | tn15b, n15b, t15b
    out_tn = nc.dram_tensor("out_tn", [1, NOUT], f32, kind="ExternalOutput")

    # plane pairs interleaved per partition (8KB descriptors)
    t_r2 = t_in.ap().rearrange("(g c) (p a) w -> g p c (a w)", c=2, p=P)
    n_r2 = n_in.ap().rearrange("(g c) (p a) w -> g p c (a w)", c=2, p=P)
    # single planes: [C, P, FREE]
    t_rs = t_in.ap().rearrange("c (p a) w -> c p (a w)", p=P)
    n_rs = n_in.ap().rearrange("c (p a) w -> c p (a w)", p=P)
    # half planes: [C, 2, P, HALF] (4KB descriptors)
    t_rh = t_in.ap().rearrange("c (h p a) w -> c h p (a w)", p=P, a=2)
    n_rh = n_in.ap().rearrange("c (h p a) w -> c h p (a w)", p=P, a=2)

    # singles at both ends (smaller first op, shorter drain), pairs between
    chunks = [("single", 0), ("single", 1)]
    chunks += [("pair", c0) for c0 in range(2, C - 2, 2)]
    chunks += [("single", C - 2), ("half", C - 1)]

    with tile.TileContext(nc) as tc:
        with (
            tc.tile_pool(name="consts", bufs=1) as consts,
            tc.tile_pool(name="tp", bufs=bufs) as tp,
            tc.tile_pool(name="npool", bufs=bufs) as npool,
            tc.tile_pool(name="sp", bufs=2) as sp,
            tc.tile_pool(name="spa", bufs=2) as spa,
            tc.tile_pool(name="outp", bufs=1) as outp,
            tc.tile_pool(name="psum", bufs=1, space="PSUM") as psum,
        ):
            ones = consts.tile([P, 1], f32)
            nc.vector.memset(ones[:], 1.0)
            acc = consts.tile([P, NOUT], f32)
            ps_fin = psum.tile([1, NOUT], f32)

            def reduce_plane(tt, nt, fsl, flen, j_tn, j_n, j_t, t_on_dve):
                # DVE: fused t*n product + per-partition rowsum
                sc = sp.tile([P, flen], bf16)
                nc.vector.scalar_tensor_tensor(
                    out=sc[:],
                    in0=tt[:, fsl],
                    scalar=1.0,
                    in1=nt[:, fsl],
                    op0=mybir.AluOpType.mult,
                    op1=mybir.AluOpType.mult,
                    accum_out=acc[:, j_tn : j_tn + 1],
                )
                # ACT: accumulating rowsum of n
                scn = spa.tile([P, flen], bf16, name="scn")
                nc.scalar.activation(
                    scn[:],
                    nt[:, fsl],
                    mybir.ActivationFunctionType.Copy,
                    accum_out=acc[:, j_n : j_n + 1],
                )
                if t_on_dve:
                    # drain tail: put t's rowsum on DVE so ACT and DVE
                    # finish the last half-plane in parallel
                    nc.vector.reduce_sum(
                        acc[:, j_t : j_t + 1], tt[:, fsl], axis=mybir.AxisListType.X
                    )
                else:
                    sct = spa.tile([P, flen], bf16, name="sct")
                    nc.scalar.activation(
                        sct[:],
                        tt[:, fsl],
                        mybir.ActivationFunctionType.Copy,
                        accum_out=acc[:, j_t : j_t + 1],
                    )

            for kind, c0 in chunks:
                # SWDGE DMAs cast f32 -> bf16 in flight (target is 0/1 so
                # exact; net_out's sums pick up ~1e-6 rel error).
                if kind == "pair":
                    tt = tp.tile([P, 2 * FREE], bf16)
                    nt = npool.tile([P, 2 * FREE], bf16)
                    nc.gpsimd.dma_start(
                        tt[:].rearrange("p (c f) -> p c f", c=2), t_r2[c0 // 2]
                    )
                    nc.gpsimd.dma_start(
                        nt[:].rearrange("p (c f) -> p c f", c=2), n_r2[c0 // 2]
                    )
                    for cc in range(2):
                        c = c0 + cc
                        reduce_plane(
                            tt, nt, slice(cc * FREE, (cc + 1) * FREE), FREE,
                            c, C + c, 2 * C + c, False,
                        )
                elif kind == "single":
                    tt = tp.tile([P, FREE], bf16)
                    nt = npool.tile([P, FREE], bf16)
                    if c0 == 0:
                        # stripe the very first op over partition blocks so
                        # the first descriptors hit the engines early
                        for q in range(4):
                            psl = slice(32 * q, 32 * (q + 1))
                            nc.gpsimd.dma_start(tt[psl, :], t_rs[0, psl])
                    else:
                        nc.gpsimd.dma_start(tt[:], t_rs[c0])
                    nc.gpsimd.dma_start(nt[:], n_rs[c0])
                    reduce_plane(
                        tt, nt, slice(0, FREE), FREE, c0, C + c0, 2 * C + c0, False
                    )
                else:  # "half": last plane in two half-plane tiles
                    for h in range(2):
                        tt = tp.tile([P, HALF], bf16)
                        nt = npool.tile([P, HALF], bf16)
                        nc.gpsimd.dma_start(tt[:], t_rh[c0, h])
                        nc.gpsimd.dma_start(nt[:], n_rh[c0, h])
                        if h == 0:
                            reduce_plane(
                                tt, nt, slice(0, HALF), HALF,
                                c0, C + c0, 2 * C + c0, False,
                            )
                        else:
                            reduce_plane(
                                tt, nt, slice(0, HALF), HALF,
                                3 * C, 3 * C + 1, 3 * C + 2, True,
                            )

            # partition-axis total: [128, NOUT] -> [1, NOUT]
            nc.tensor.matmul(ps_fin[:, :], ones[:], acc[:], start=True, stop=True)
            sb = outp.tile([1, NOUT], f32)
            nc.vector.tensor_copy(sb[:], ps_fin[:])
            nc.sync.dma_start(out_tn.ap(), sb[:])

    nc.compile()
    return nc


def _build_f32(C=C, H=H, W=W, num_devices=NCORES, bufs=3):
    """All-f32 variant: HWDGE loads (no cast), no TensorE in the main loop.
    DVE: fused mul+rowsum of t*n, plus rowsum of t.  ACT: accumulating
    rowsums of n and m.  One final f32 matmul reduces the [128, 4C]
    accumulator across partitions."""
    import concourse.bacc as bacc
    import concourse.mybir as mybir
    import concourse.tile as tile

    P = 128
    FREE = H * W // P

    f32 = mybir.dt.float32
    nc = bacc.Bacc(
        "TRN2", target_bir_lowering=False, debug=False, num_devices=num_devices
    )

    t_in = nc.dram_tensor("t_in", [C, H, W], f32, kind="ExternalInput")
    n_in = nc.dram_tensor("n_in", [C, H, W], f32, kind="ExternalInput")
    m_in = nc.dram_tensor("m_in", [C, H, W], f32, kind="ExternalInput")
    out_tn = nc.dram_tensor("out_tn", [1, 4 * C], f32, kind="ExternalOutput")

    t_r = t_in.ap().rearrange("c (p a) w -> c p (a w)", p=P)
    n_r = n_in.ap().rearrange("c (p a) w -> c p (a w)", p=P)
    m_r = m_in.ap().rearrange("c (p a) w -> c p (a w)", p=P)

    with tile.TileContext(nc) as tc:
        with (
            tc.tile_pool(name="consts", bufs=1) as consts,
            tc.tile_pool(name="tp", bufs=bufs) as tp,
            tc.tile_pool(name="npool", bufs=bufs) as npool,
            tc.tile_pool(name="mp", bufs=bufs) as mp,
            tc.tile_pool(name="sp", bufs=2) as sp,
            tc.tile_pool(name="spa", bufs=2) as spa,
            tc.tile_pool(name="outp", bufs=1) as outp,
            tc.tile_pool(name="psum", bufs=1, space="PSUM") as psum,
        ):
            ones = consts.tile([P, 1], f32)
            nc.vector.memset(ones[:], 1.0)
            # cols [0,C)=t*n  [C,2C)=m  [2C,3C)=t  [3C,4C)=n
            acc = consts.tile([P, 4 * C], f32)
            ps_fin = psum.tile([1, 4 * C], f32)

            for c in range(C):
                tt = tp.tile([P, FREE], f32, name="tt")
                nc.sync.dma_start(tt[:], t_r[c])
                nt = npool.tile([P, FREE], f32, name="nt")
                nc.scalar.dma_start(nt[:], n_r[c])
                mt = mp.tile([P, FREE], f32, name="mt")
                nc.sync.dma_start(mt[:], m_r[c])

                sc = sp.tile([P, FREE], f32, name="sc")
                nc.vector.scalar_tensor_tensor(
                    out=sc[:],
                    in0=tt[:],
                    scalar=1.0,
                    in1=nt[:],
                    op0=mybir.AluOpType.mult,
                    op1=mybir.AluOpType.mult,
                    accum_out=acc[:, c : c + 1],
                )
                nc.vector.reduce_sum(
                    acc[:, 2 * C + c : 2 * C + c + 1],
                    tt[:],
                    axis=mybir.AxisListType.X,
                )
                scn = spa.tile([P, FREE], f32, name="scn")
                nc.scalar.activation(
                    scn[:],
                    nt[:],
                    mybir.ActivationFunctionType.Copy,
                    accum_out=acc[:, 3 * C + c : 3 * C + c + 1],
                )
                scm = spa.tile([P, FREE], f32, name="scm")
                nc.scalar.activation(
                    scm[:],
                    mt[:],
                    mybir.ActivationFunctionType.Copy,
                    accum_out=acc[:, C + c : C + c + 1],
                )

            nc.tensor.matmul(ps_fin[:, :], ones[:], acc[:], start=True, stop=True)
            sb = outp.tile([1, 4 * C], f32)
            nc.vector.tensor_copy(sb[:], ps_fin[:])
            nc.sync.dma_start(out_tn.ap(), sb[:])

    nc.compile()
    return nc


_V = os.environ.get("K_V", "5")
_M_ON = os.environ.get("K_M_ON", "skip")
_N_ON = os.environ.get("K_N_ON", "act")
_BUFS = int(os.environ.get("K_BUFS", "5"))
_CPT = int(os.environ.get("K_CPT", "2"))
_EDGE = os.environ.get("K_EDGE", "1") == "1"


def _get_nc():
    if "nc" not in _CACHE:
        if _V == "5":
            _CACHE["nc"] = _build_v5(bufs=_BUFS)
        elif _V == "4":
            _CACHE["nc"] = _build_v4(bufs=_BUFS)
        else:
            _CACHE["nc"] = _build(
                m_on=_M_ON,
                n_on=_N_ON,
                bufs=_BUFS,
                cpt=_CPT,
                edge_split=_EDGE,
                m_f32=os.environ.get("K_MF32", "0") == "1",
            )
    return _CACHE["nc"]


def _run(net_out, target, max_positiones, trace=False):
    from concourse.bass_utils import run_bass_kernel_spmd

    nc = _get_nc()
    in_maps = []
    for i in range(NCORES):
        im = {
            "t_in": np.ascontiguousarray(target[i]),
            "n_in": np.ascontiguousarray(net_out[i]),
        }
        if (_V not in ("4", "5") and _M_ON != "skip") or (
            _V == "5" and os.environ.get("K_PROBE_HW", "0") != "0"
        ):
            im["m_in"] = np.ascontiguousarray(max_positiones[i])
        in_maps.append(im)
    res = run_bass_kernel_spmd(
        nc, in_maps, core_ids=list(range(NCORES)), trace=trace
    )
    return res


def _finish(results, max_positiones=None, n_in_tn=False):
    # results: list (per core) of {"out_tn": [1,2C] (tn | m-or-n sums),
    #                               "out_tnm": [C,3] (t, n, m; unused cols 0)}
    tnm_flat = np.stack([r["out_tn"][0] for r in results]).astype(np.float64)
    if tnm_flat.shape[1] in (34, 35):  # v5/v6: tn[C] | n[C] | tn15b, n15b[, t0]
        tn = tnm_flat[:, 0:16].copy()
        sn = tnm_flat[:, 16:32].copy()
        tn[:, 15] += tnm_flat[:, 32]
        sn[:, 15] += tnm_flat[:, 33]
        st = np.stack([r["out_tnm"][:, 0] for r in results]).astype(np.float64)
        if tnm_flat.shape[1] == 35:  # head plane's t summed on ACT, not PE
            st[:, 0] = tnm_flat[:, 34]
        sm = np.zeros_like(st)
    elif tnm_flat.shape[1] == 51:  # v4: tn[C] | n[C] | t[C] | tn15b,n15b,t15b
        tn = tnm_flat[:, 0:16].copy()
        sn = tnm_flat[:, 16:32].copy()
        st = tnm_flat[:, 32:48].copy()
        tn[:, 15] += tnm_flat[:, 48]
        sn[:, 15] += tnm_flat[:, 49]
        st[:, 15] += tnm_flat[:, 50]
        sm = np.zeros_like(st)
    elif tnm_flat.shape[1] == 64:  # all-f32 layout: tn | m | t | n
        tn, sm = tnm_flat[:, :16], tnm_flat[:, 16:32]
        st, sn = tnm_flat[:, 32:48], tnm_flat[:, 48:64]
    elif n_in_tn:  # skip mode with n on ACT: out_tn = tn | n, out_tnm col0 = t
        tn, sn = tnm_flat[:, :16], tnm_flat[:, 16:]
        tnm = np.stack([r["out_tnm"] for r in results]).astype(np.float64)
        st = tnm[..., 0]
        sm = np.zeros_like(st)
    else:
        tn, sm_a = tnm_flat[:, :16], tnm_flat[:, 16:]  # [B,C] each
        tnm = np.stack([r["out_tnm"] for r in results]).astype(np.float64)
        st, sn, sm_b = tnm[..., 0], tnm[..., 1], tnm[..., 2]
        sm = sm_a + sm_b  # exactly one of the two paths populated its slot

    b2 = 1.5 * 1.5
    w1 = b2 / (1.0 + b2)
    w2 = 1.0 / (1.0 + b2)
    molecule = tn
    fn = st - tn
    fp = sn - tn
    loss = 1.0 - molecule / (molecule + w1 * fn + w2 * fp)
    active = (st > 0) | (sm > 0)
    if max_positiones is not None:
        # device skipped max_positiones (it only matters for planes whose
        # target is entirely zero); resolve those few planes on host.
        for b, c in zip(*np.nonzero(~active)):
            if np.max(max_positiones[b, c]) > 0:
                active[b, c] = True
    losses = np.where(active, loss, 0.0)
    cnt = np.sum(losses != 0, axis=1).astype(np.float64)
    img_losses = np.sum(losses, axis=1) / cnt
    out = np.sum(img_losses) / img_losses.shape[0]
    return np.asarray(out, dtype=np.float32)


def kernel(net_out, target, max_positiones):
    net_out = np.asarray(net_out, dtype=np.float32)
    target = np.asarray(target, dtype=np.float32)
    max_positiones = np.asarray(max_positiones, dtype=np.float32)
    res = _run(net_out, target, max_positiones, trace=False)
    return _finish(
        res.results,
        max_positiones if (_M_ON == "skip" or _V in ("4", "5")) else None,
        n_in_tn=(_N_ON == "act" and _M_ON == "skip"),
    )



# revision 46
# speedup vs baseline: 1.0574x; 1.0574x over previous
"""Tversky-style mismatch loss on Trainium2 (Bass/Tile), 8-core data-parallel.

Full inputs: net_out/target/max_positiones, each [8, 16, 512, 512] f32.
Sharding: batch dim B=8 across 8 NeuronCores (1 image per core).

Per (image, class) plane only tn = sum(target*net_out), t = sum(target)
and n = sum(net_out) are needed on-device, since fn = t - tn, fp = n - tn.
max_positiones is NEVER loaded: its only use is active = (max t)|(max m)>0,
and the m term is dead unless a target plane is entirely zero -- the host
resolves those (rare-to-nonexistent) planes from the returned t sums by
scanning max_positiones in numpy.  That cuts device HBM traffic from 48 to
32 MiB/core (137.7us -> ~95us; the per-core DMA engine pool sustains ~420
GB/s read, which the trace shows is shared across SWDGE and HWDGE queues,
so 32 MiB ~= 80us of streaming is the hardware floor).

Engine split, all under the stream cadence: DVE scalar_tensor_tensor does
the fused t*n product + per-partition rowsum, ACT (Copy + accum_out) sums
n, PE sums t's columns via a shifted ones-window lhsT so each plane's
column sums accumulate into its own PSUM row.  A final ones-matmul reduces
the [128, NOUT] per-partition accumulator across partitions.  Inputs are
cast f32->bf16 in-flight by the SWDGE DMA (target is 0/1 so exact;
net_out's sums pick up ~1e-6 rel err; overall rel err 5e-7).  Plane 0 goes
f32 over the two HWDGE queues, which start ~2us before the SWDGE ucode
spins up (the engine pool is the shared cap, so starting earlier ends
earlier); the last plane is split into half-planes to shorten the
post-stream drain.  The two tiny outputs leave on different queues.
bufs=5 keeps ~5 tile-sets of prefetch against the machine's intermittent
contended windows (runs land ~95us clean, ~115us when an engine's HBM
path is contended by other tenants).
The tiny [8,16] -> scalar tail runs on host in float64.
"""

import os
import sys

import numpy as np

if "/opt/trn_rl_repo" not in sys.path:
    sys.path.insert(0, "/opt/trn_rl_repo")

B, C, H, W = 8, 16, 512, 512
NCORES = 8
P = 128
FREE = H * W // P  # 2048 f32 per partition per plane
CHUNK = 512  # max fp32 moving free dim per matmul
NCHUNK = FREE // CHUNK  # 4

_CACHE = {}


def _build(C=C, H=H, W=W, debug=False, num_devices=NCORES, m_on="act", bufs=3, cpt=2, m_f32=False, split_first=False, n_on="pe", edge_split=False):
    import concourse.bacc as bacc
    import concourse.mybir as mybir
    import concourse.tile as tile

    P = 128
    FREE = H * W // P
    CHUNK = min(512, FREE)
    NCHUNK = FREE // CHUNK

    f32 = mybir.dt.float32
    bf16 = mybir.dt.bfloat16
    nc = bacc.Bacc(
        "TRN2", target_bir_lowering=False, debug=debug, num_devices=num_devices
    )

    t_in = nc.dram_tensor("t_in", [C, H, W], f32, kind="ExternalInput")
    n_in = nc.dram_tensor("n_in", [C, H, W], f32, kind="ExternalInput")
    m_in = (
        nc.dram_tensor("m_in", [C, H, W], f32, kind="ExternalInput")
        if m_on != "skip"
        else None
    )
    out_tn = nc.dram_tensor("out_tn", [1, 2 * C], f32, kind="ExternalOutput")
    out_tnm = nc.dram_tensor("out_tnm", [C, 3], f32, kind="ExternalOutput")

    # pair of planes g as [128 partitions, 2 x 2048 contiguous f32]
    CPT = cpt if C % cpt == 0 else 1  # planes per DMA tile
    NT = C // CPT
    t_r = t_in.ap().rearrange("(g c) (p a) w -> g p c (a w)", c=CPT, p=P)
    n_r = n_in.ap().rearrange("(g c) (p a) w -> g p c (a w)", c=CPT, p=P)
    m_r = (
        m_in.ap().rearrange("(g c) (p a) w -> g p c (a w)", c=CPT, p=P)
        if m_in is not None
        else None
    )
    # per-plane views for edge (head/tail) chunks
    t_r1 = t_in.ap().rearrange("(g c) (p a) w -> g p c (a w)", c=1, p=P)
    n_r1 = n_in.ap().rearrange("(g c) (p a) w -> g p c (a w)", c=1, p=P)

    if edge_split and m_on == "skip" and CPT == 2 and C >= 6:
        # single planes at both ends: first DMA op has fewer descriptors
        # (stream starts sooner) and the last tiles shrink the drain tail.
        chunks = [(0, 1), (1, 1)]
        chunks += [(c0, 2) for c0 in range(2, C - 2, 2)]
        chunks += [(C - 2, 1), (C - 1, 1)]
    else:
        chunks = [(g * CPT, CPT) for g in range(NT)]

    with tile.TileContext(nc) as tc:
        with (
            tc.tile_pool(name="consts", bufs=1) as consts,
            tc.tile_pool(name="tp", bufs=bufs) as tp,
            tc.tile_pool(name="npool", bufs=bufs) as npool,
            tc.tile_pool(name="mp", bufs=bufs) as mp,
            tc.tile_pool(name="sp", bufs=2) as sp,
            tc.tile_pool(name="spa", bufs=2) as spa,
            tc.tile_pool(name="outp", bufs=1) as outp,
            tc.tile_pool(name="psum", bufs=1, space="PSUM") as psum,
        ):
            ones = consts.tile([P, 1], f32)
            nc.vector.memset(ones[:], 1.0)
            # G[:, C-1] = 1, rest 0.  lhsT window G[:, C-1-c : 2C-1-c] is a
            # [P, C] matrix whose column c is all-ones -> plane c's column
            # sums land in PSUM partition row c, other rows accumulate +0.
            G = consts.tile([P, 2 * C - 1], bf16)
            nc.vector.memset(G[:], 0.0)
            nc.vector.memset(G[:, C - 1 : C], 1.0)
            # per-plane partition-partials: cols [0,C) = t*n, cols [C,2C) = m
            acc = consts.tile([P, 2 * C], f32)
            if m_on in ("pe", "skip"):
                nc.vector.memset(acc[:, C:], 0.0)  # m half unused

            ps_t = psum.tile([C, CHUNK], f32)
            ps_n = psum.tile([C, CHUNK], f32) if n_on == "pe" else None
            ps_m = psum.tile([C, CHUNK], f32, name="ps_m") if m_on == "pe" else None
            ps_tn = psum.tile([1, 2 * C], f32)

            for c0, ncpl in chunks:
                # SWDGE DMAs cast f32 -> bf16 in flight (HWDGE can't cast).
                # target/max_positiones are 0/1-valued so bf16 is exact;
                # net_out's per-plane sums only pick up ~1e-6 rel error.
                tt = tp.tile([P, ncpl * FREE], bf16)
                nt = npool.tile([P, ncpl * FREE], bf16)
                if ncpl == CPT:
                    g = c0 // CPT
                    if split_first and c0 == 0:
                        for q in range(CPT):
                            nc.gpsimd.dma_start(
                                tt[:, q * FREE : (q + 1) * FREE], t_r[g, :, q]
                            )
                            nc.gpsimd.dma_start(
                                nt[:, q * FREE : (q + 1) * FREE], n_r[g, :, q]
                            )
                    else:
                        nc.gpsimd.dma_start(
                            tt[:].rearrange("p (c f) -> p c f", c=CPT), t_r[g]
                        )
                        nc.gpsimd.dma_start(
                            nt[:].rearrange("p (c f) -> p c f", c=CPT), n_r[g]
                        )
                else:
                    assert ncpl == 1
                    nc.gpsimd.dma_start(
                        tt[:].rearrange("p (c f) -> p c f", c=1), t_r1[c0]
                    )
                    nc.gpsimd.dma_start(
                        nt[:].rearrange("p (c f) -> p c f", c=1), n_r1[c0]
                    )
                if m_on == "skip":
                    mt = None
                elif m_f32:
                    # m feeds only the ACT accumulator; load it f32 over
                    # HWDGE to take work off the single SWDGE queue.
                    mt = mp.tile([P, ncpl * FREE], f32, name="mt")
                    nc.sync.dma_start(
                        mt[:].rearrange("p (c f) -> p c f", c=ncpl), m_r[c0 // CPT]
                    )
                else:
                    mt = mp.tile([P, ncpl * FREE], bf16, name="mt")
                    nc.gpsimd.dma_start(
                        mt[:].rearrange("p (c f) -> p c f", c=ncpl), m_r[c0 // CPT]
                    )

                for cc in range(ncpl):
                    c = c0 + cc
                    fsl = slice(cc * FREE, (cc + 1) * FREE)
                    # DVE: fused product + per-partition row sum of t*n.
                    sc = sp.tile([P, FREE], bf16)
                    nc.vector.scalar_tensor_tensor(
                        out=sc[:],
                        in0=tt[:, fsl],
                        scalar=1.0,
                        in1=nt[:, fsl],
                        op0=mybir.AluOpType.mult,
                        op1=mybir.AluOpType.mult,
                        accum_out=acc[:, c : c + 1],
                    )
                    if m_on == "act":
                        # ACT (own SBUF port, otherwise idle): accumulating
                        # sum of m into the accumulator's second half.
                        scm = spa.tile([P, FREE], bf16, name="scm")
                        nc.scalar.activation(
                            scm[:],
                            mt[:, fsl],
                            mybir.ActivationFunctionType.Copy,
                            accum_out=acc[:, C + c : C + c + 1],
                        )
                    if n_on == "act":
                        # ACT (otherwise idle in skip mode): accumulating sum
                        # of net_out into the accumulator's second half, so
                        # PE only carries target's column sums.
                        scn = spa.tile([P, FREE], bf16, name="scn")
                        nc.scalar.activation(
                            scn[:],
                            nt[:, fsl],
                            mybir.ActivationFunctionType.Copy,
                            accum_out=acc[:, C + c : C + c + 1],
                        )

                    w = G[:, C - 1 - c : 2 * C - 1 - c]
                    for k in range(NCHUNK):
                        first = c == 0 and k == 0
                        last = c == C - 1 and k == NCHUNK - 1
                        sl = slice(cc * FREE + k * CHUNK, cc * FREE + (k + 1) * CHUNK)
                        nc.tensor.matmul(
                            ps_t[:, :], w, tt[:, sl], start=first, stop=last
                        )
                        if n_on == "pe":
                            nc.tensor.matmul(
                                ps_n[:, :], w, nt[:, sl], start=first, stop=last
                            )
                        if m_on == "pe":
                            nc.tensor.matmul(
                                ps_m[:, :], w, mt[:, sl], start=first, stop=last
                            )

            # partition-axis total of the tn/m partials: [128, 2C] -> [1, 2C]
            nc.tensor.matmul(ps_tn[:, :], ones[:], acc[:], start=True, stop=True)

            sb_tnm = outp.tile([C, 3], f32)
            nc.vector.reduce_sum(sb_tnm[:, 0:1], ps_t[:], axis=mybir.AxisListType.X)
            if n_on == "pe":
                nc.vector.reduce_sum(
                    sb_tnm[:, 1:2], ps_n[:], axis=mybir.AxisListType.X
                )
            else:
                nc.vector.memset(sb_tnm[:, 1:2], 0.0)  # n sums live in acc
            if m_on == "pe":
                nc.vector.reduce_sum(
                    sb_tnm[:, 2:3], ps_m[:], axis=mybir.AxisListType.X
                )
            else:
                nc.vector.memset(sb_tnm[:, 2:3], 0.0)  # unused in act mode
            sb_tn = outp.tile([1, 2 * C], f32)
            nc.vector.tensor_copy(sb_tn[:], ps_tn[:])

            nc.sync.dma_start(out_tnm.ap(), sb_tnm[:])
            nc.sync.dma_start(out_tn.ap(), sb_tn[:])

    nc.compile()
    return nc


def _build_v7(C=C, H=H, W=W, num_devices=NCORES, bufs=5):
    """v6 plus pair-packed middle chunks: planes (2g, 2g+1) load as one
    [1024, 512] block whose partitions are 8 contiguous rows = 16KB
    descriptors (624ns vs 2x323ns per 16KB read -> ~3.4% more stream
    bandwidth on the shared engine pool).  Even plane lands in partitions
    0-63, odd in 64-127, so the final partition-reduce uses a [P, 3] ones
    matrix (all | upper | lower) producing [3, NOUT]: full-partition sums in
    row 0 (edge planes), per-half sums in rows 1-2 (paired planes).  PE's
    column sums use a second shifted window G2 whose two hot columns are
    half-ones, steering each half-plane into its own PSUM row."""
    import concourse.bacc as bacc
    import concourse.mybir as mybir
    import concourse.tile as tile

    P = 128
    FREE = H * W // P  # 2048
    HALF = FREE // 2
    CHUNK = min(512, FREE)
    NCHUNK = FREE // CHUNK
    NPAIR = (C - 4) // 2  # planes 2..13 pair-packed

    f32 = mybir.dt.float32
    bf16 = mybir.dt.bfloat16
    nc = bacc.Bacc(
        "TRN2", target_bir_lowering=False, debug=False, num_devices=num_devices
    )

    t_in = nc.dram_tensor("t_in", [C, H, W], f32, kind="ExternalInput")
    n_in = nc.dram_tensor("n_in", [C, H, W], f32, kind="ExternalInput")
    # acc columns: [0,6) tn pairs | [6,12) n pairs | 12,13 tn/n plane1 |
    # 14,15 tn/n plane14 | 16,17 tn/n 15a | 18,19 tn/n 15b | 20,21,22 plane0
    NOUT = 2 * NPAIR + 11  # 23
    out_tn = nc.dram_tensor("out_tn", [3, NOUT], f32, kind="ExternalOutput")
    out_tnm = nc.dram_tensor("out_tnm", [C, 1], f32, kind="ExternalOutput")

    t_rs = t_in.ap().rearrange("c (p a) w -> c p (a w)", p=P)
    n_rs = n_in.ap().rearrange("c (p a) w -> c p (a w)", p=P)
    t_rh = t_in.ap().rearrange("c (h p a) w -> c h p (a w)", p=P, a=2)
    n_rh = n_in.ap().rearrange("c (h p a) w -> c h p (a w)", p=P, a=2)
    t_r16 = t_in.ap().rearrange("(g c) (p2 a) w -> g (c p2) (a w)", c=2, p2=P // 2)
    n_r16 = n_in.ap().rearrange("(g c) (p2 a) w -> g (c p2) (a w)", c=2, p2=P // 2)

    with tile.TileContext(nc) as tc:
        with (
            tc.tile_pool(name="consts", bufs=1) as consts,
            tc.tile_pool(name="tp", bufs=bufs) as tp,
            tc.tile_pool(name="npool", bufs=bufs) as npool,
            tc.tile_pool(name="sp", bufs=2) as sp,
            tc.tile_pool(name="spa", bufs=2) as spa,
            tc.tile_pool(name="outp", bufs=1) as outp,
            tc.tile_pool(name="psum", bufs=1, space="PSUM") as psum,
        ):
            # O: col0 = all ones, col1 = upper-half ones, col2 = lower-half
            O = consts.tile([P, 3], f32)
            nc.vector.memset(O[:], 0.0)
            nc.vector.memset(O[:, 0:1], 1.0)
            nc.vector.memset(O[0 : P // 2, 1:2], 1.0)
            nc.vector.memset(O[P // 2 : P, 2:3], 1.0)
            # G: full-partition ones window (edge planes)
            G = consts.tile([P, 2 * C - 1], bf16)
            nc.vector.memset(G[:], 0.0)
            nc.vector.memset(G[:, C - 1 : C], 1.0)
            # G2: half-partition ones windows (paired planes); hot cols at
            # C-2 (upper half) and C-1 (lower half)
            G2 = consts.tile([P, 28], bf16)
            nc.vector.memset(G2[:], 0.0)
            nc.vector.memset(G2[0 : P // 2, C - 2 : C - 1], 1.0)
            nc.vector.memset(G2[P // 2 : P, C - 1 : C], 1.0)
            acc = consts.tile([P, NOUT], f32)
            ps_t = psum.tile([C, CHUNK], f32)
            ps_fin = psum.tile([3, NOUT], f32)

            def dve_act(tt, nt, fsl, flen, j_tn, j_n):
                sc = sp.tile([P, flen], bf16)
                nc.vector.scalar_tensor_tensor(
                    out=sc[:],
                    in0=tt[:, fsl],
                    scalar=1.0,
                    in1=nt[:, fsl],
                    op0=mybir.AluOpType.mult,
                    op1=mybir.AluOpType.mult,
                    accum_out=acc[:, j_tn : j_tn + 1],
                )
                scn = spa.tile([P, flen], bf16, name="scn")
                nc.scalar.activation(
                    scn[:],
                    nt[:, fsl],
                    mybir.ActivationFunctionType.Copy,
                    accum_out=acc[:, j_n : j_n + 1],
                )

            def pe_cols(tt, w, sls, first_sl, stop_last):
                for i, sl in enumerate(sls):
                    nc.tensor.matmul(
                        ps_t[:, :], w, tt[:, sl],
                        start=(first_sl and i == 0),
                        stop=(stop_last and i == len(sls) - 1),
                    )

            # ---- plane 0: f32 over the two HWDGE queues (earliest start)
            tf = tp.tile([P, FREE], f32, name="tf0")
            nf = npool.tile([P, FREE], f32, name="nf0")
            nc.sync.dma_start(tf[:], t_rs[0])
            nc.scalar.dma_start(nf[:], n_rs[0])
            sc0 = sp.tile([P, FREE], bf16)
            nc.vector.scalar_tensor_tensor(
                out=sc0[:], in0=tf[:], scalar=1.0, in1=nf[:],
                op0=mybir.AluOpType.mult, op1=mybir.AluOpType.mult,
                accum_out=acc[:, 20:21],
            )
            scn0 = spa.tile([P, FREE], bf16, name="scn")
            nc.scalar.activation(
                scn0[:], nf[:], mybir.ActivationFunctionType.Copy,
                accum_out=acc[:, 21:22],
            )
            sct0 = spa.tile([P, FREE], bf16, name="sct")
            nc.scalar.activation(
                sct0[:], tf[:], mybir.ActivationFunctionType.Copy,
                accum_out=acc[:, 22:23],
            )

            # ---- plane 1: single, full-partition layout
            tt = tp.tile([P, FREE], bf16)
            nt = npool.tile([P, FREE], bf16)
            nc.gpsimd.dma_start(tt[:], t_rs[1])
            nc.gpsimd.dma_start(nt[:], n_rs[1])
            dve_act(tt, nt, slice(0, FREE), FREE, 12, 13)
            pe_cols(
                tt, G[:, C - 2 : 2 * C - 2],
                [slice(k * CHUNK, (k + 1) * CHUNK) for k in range(NCHUNK)],
                True, False,
            )

            # ---- planes 2..13: pair-packed 16KB-descriptor chunks
            for p in range(NPAIR):
                g = p + 1  # pair-view index: planes (2g, 2g+1)
                tt = tp.tile([P, 2 * FREE], bf16)
                nt = npool.tile([P, 2 * FREE], bf16)
                nc.gpsimd.dma_start(tt[:], t_r16[g])
                nc.gpsimd.dma_start(nt[:], n_r16[g])
                dve_act(tt, nt, slice(0, 2 * FREE), 2 * FREE, p, NPAIR + p)
                pe_cols(
                    tt, G2[:, C - 4 - 2 * p : 2 * C - 4 - 2 * p],
                    [slice(k * CHUNK, (k + 1) * CHUNK) for k in range(2 * NCHUNK)],
                    False, False,
                )

            # ---- plane 14: single
            tt = tp.tile([P, FREE], bf16)
            nt = npool.tile([P, FREE], bf16)
            nc.gpsimd.dma_start(tt[:], t_rs[14])
            nc.gpsimd.dma_start(nt[:], n_rs[14])
            dve_act(tt, nt, slice(0, FREE), FREE, 14, 15)
            pe_cols(
                tt, G[:, C - 15 : 2 * C - 15],
                [slice(k * CHUNK, (k + 1) * CHUNK) for k in range(NCHUNK)],
                False, False,
            )

            # ---- plane 15: two half-planes (short drain)
            for h in range(2):
                tt = tp.tile([P, HALF], bf16)
                nt = npool.tile([P, HALF], bf16)
                nc.gpsimd.dma_start(tt[:], t_rh[15, h])
                nc.gpsimd.dma_start(nt[:], n_rh[15, h])
                dve_act(tt, nt, slice(0, HALF), HALF, 16 + 2 * h, 17 + 2 * h)
                pe_cols(
                    tt, G[:, C - 16 : 2 * C - 16],
                    [slice(k * CHUNK, (k + 1) * CHUNK) for k in range(HALF // CHUNK)],
                    False, h == 1,
                )

            sb_t = outp.tile([C, 1], f32)
            nc.vector.reduce_sum(sb_t[:], ps_t[:], axis=mybir.AxisListType.X)
            nc.tensor.matmul(ps_fin[:, :], O[:], acc[:], start=True, stop=True)
            sb = outp.tile([3, NOUT], f32)
            nc.vector.tensor_copy(sb[:], ps_fin[:])
            nc.scalar.dma_start(out_tnm.ap(), sb_t[:])
            nc.sync.dma_start(out_tn.ap(), sb[:])

    nc.compile()
    return nc


def _build_v5(C=C, H=H, W=W, num_devices=NCORES, bufs=5):
    """v3 engine split (DVE: tn rowsums, ACT: n rowsums, PE: t column sums
    via the shifted ones-window trick) with a half-plane split of the last
    plane to shorten the post-stream drain, and the two output DMAs on
    different queues so they overlap.  m (max_positiones) is never loaded:
    it only matters for planes whose target is entirely zero, which the
    host resolves from the returned t sums."""
    import concourse.bacc as bacc
    import concourse.mybir as mybir
    import concourse.tile as tile

    P = 128
    FREE = H * W // P  # 2048
    HALF = FREE // 2
    CHUNK = min(512, FREE)
    NCHUNK = FREE // CHUNK

    f32 = mybir.dt.float32
    bf16 = mybir.dt.bfloat16
    nc = bacc.Bacc(
        "TRN2", target_bir_lowering=False, debug=False, num_devices=num_devices
    )

    t_in = nc.dram_tensor("t_in", [C, H, W], f32, kind="ExternalInput")
    n_in = nc.dram_tensor("n_in", [C, H, W], f32, kind="ExternalInput")
    # plane 0 goes f32 over the two HWDGE queues, which start ~2us before
    # the SWDGE ucode spins up; the engine pool is the shared cap, so
    # starting earlier ends the stream earlier.
    head_f32 = os.environ.get("K_HEADF32", "1") == "1"
    NOUT = 2 * C + 3  # tn[C] | n[C] | tn15b, n15b, t0
    out_tn = nc.dram_tensor("out_tn", [1, NOUT], f32, kind="ExternalOutput")
    out_tnm = nc.dram_tensor("out_tnm", [C, 1], f32, kind="ExternalOutput")

    # bandwidth probe: stream some never-used f32 planes of m over the
    # sync HWDGE queue to test whether HW queues add DMA bandwidth
    probe_hw = int(os.environ.get("K_PROBE_HW", "0"))
    m_in = (
        nc.dram_tensor("m_in", [C, H, W], f32, kind="ExternalInput")
        if probe_hw
        else None
    )

    t_r2 = t_in.ap().rearrange("(g c) (p a) w -> g p c (a w)", c=2, p=P)
    n_r2 = n_in.ap().rearrange("(g c) (p a) w -> g p c (a w)", c=2, p=P)
    t_rs = t_in.ap().rearrange("c (p a) w -> c p (a w)", p=P)
    n_rs = n_in.ap().rearrange("c (p a) w -> c p (a w)", p=P)
    t_rh = t_in.ap().rearrange("c (h p a) w -> c h p (a w)", p=P, a=2)
    n_rh = n_in.ap().rearrange("c (h p a) w -> c h p (a w)", p=P, a=2)
    m_rs = (
        m_in.ap().rearrange("c (p a) w -> c p (a w)", p=P)
        if m_in is not None
        else None
    )
    # pair-packed view: planes (2g, 2g+1) as [1024, 512]; partition = 8
    # contiguous rows = 16KB contiguous per descriptor (vs 8KB normally)
    m_r16 = (
        m_in.ap().rearrange("(g c) (p2 a) w -> g (c p2) (a w)", c=2, p2=P // 2)
        if m_in is not None
        else None
    )

    chunks = [("single", 0), ("single", 1)]
    chunks += [("pair", c0) for c0 in range(2, C - 2, 2)]
    chunks += [("single", C - 2), ("half", C - 1)]

    with tile.TileContext(nc) as tc:
        with (
            tc.tile_pool(name="consts", bufs=1) as consts,
            tc.tile_pool(name="tp", bufs=bufs) as tp,
            tc.tile_pool(name="npool", bufs=bufs) as npool,
            tc.tile_pool(name="sp", bufs=2) as sp,
            tc.tile_pool(name="spa", bufs=2) as spa,
            tc.tile_pool(name="outp", bufs=1) as outp,
            tc.tile_pool(name="psum", bufs=1, space="PSUM") as psum,
        ):
            ones = consts.tile([P, 1], f32)
            nc.vector.memset(ones[:], 1.0)
            # G[:, C-1] = 1, rest 0: lhsT window G[:, C-1-c : 2C-1-c] is a
            # [P, C] matrix whose column c is all-ones -> plane c's column
            # sums land in PSUM row c, other rows accumulate +0.
            G = consts.tile([P, 2 * C - 1], bf16)
            nc.vector.memset(G[:], 0.0)
            nc.vector.memset(G[:, C - 1 : C], 1.0)
            acc = consts.tile([P, NOUT], f32)
            ps_t = psum.tile([C, CHUNK], f32)
            ps_tn = psum.tile([1, NOUT], f32)

            def reduce_part(tt, nt, fsl, flen, j_tn, j_n, c, mm_sls, stop):
                # DVE: fused t*n product + per-partition rowsum
                sc = sp.tile([P, flen], bf16)
                nc.vector.scalar_tensor_tensor(
                    out=sc[:],
                    in0=tt[:, fsl],
                    scalar=1.0,
                    in1=nt[:, fsl],
                    op0=mybir.AluOpType.mult,
                    op1=mybir.AluOpType.mult,
                    accum_out=acc[:, j_tn : j_tn + 1],
                )
                # ACT: accumulating rowsum of n
                scn = spa.tile([P, flen], bf16, name="scn")
                nc.scalar.activation(
                    scn[:],
                    nt[:, fsl],
                    mybir.ActivationFunctionType.Copy,
                    accum_out=acc[:, j_n : j_n + 1],
                )
                # PE: t's column sums accumulate into ps_t row c
                w = G[:, C - 1 - c : 2 * C - 1 - c]
                for i, sl in enumerate(mm_sls):
                    first = c == (1 if head_f32 else 0) and sl.start == 0
                    nc.tensor.matmul(
                        ps_t[:, :], w, tt[:, sl], start=first,
                        stop=stop and i == len(mm_sls) - 1,
                    )

            probes_issued = 0

            for kind, c0 in chunks:
                if head_f32 and c0 == 0:
                    # plane 0: f32 over both HWDGE queues (earliest start);
                    # its three reductions run engine-side on the f32 tiles
                    # (DVE f32 is 2x slower but overlaps the long stream).
                    tf = tp.tile([P, FREE], f32, name="tf0")
                    nf = npool.tile([P, FREE], f32, name="nf0")
                    nc.sync.dma_start(tf[:], t_rs[0])
                    nc.scalar.dma_start(nf[:], n_rs[0])
                    sc = sp.tile([P, FREE], bf16)
                    nc.vector.scalar_tensor_tensor(
                        out=sc[:],
                        in0=tf[:],
                        scalar=1.0,
                        in1=nf[:],
                        op0=mybir.AluOpType.mult,
                        op1=mybir.AluOpType.mult,
                        accum_out=acc[:, 0:1],
                    )
                    scn = spa.tile([P, FREE], bf16, name="scn")
                    nc.scalar.activation(
                        scn[:],
                        nf[:],
                        mybir.ActivationFunctionType.Copy,
                        accum_out=acc[:, C : C + 1],
                    )
                    sct = spa.tile([P, FREE], bf16, name="sct")
                    nc.scalar.activation(
                        sct[:],
                        tf[:],
                        mybir.ActivationFunctionType.Copy,
                        accum_out=acc[:, 2 * C + 2 : 2 * C + 3],
                    )
                    continue
                if probe_hw and probes_issued < probe_hw and c0 % 2 == 0:
                    # 16KB-descriptor timing probe: pair-packed bf16 load
                    mt = outp.tile([P, 2 * FREE], bf16, name="mt_probe")
                    nc.gpsimd.dma_start(mt[:], m_r16[probes_issued])
                    probes_issued += 1
                # SWDGE DMAs cast f32 -> bf16 in flight (target is 0/1 so
                # exact; net_out's sums pick up ~1e-6 rel error).
                if kind == "pair":
                    tt = tp.tile([P, 2 * FREE], bf16)
                    nt = npool.tile([P, 2 * FREE], bf16)
                    nc.gpsimd.dma_start(
                        tt[:].rearrange("p (c f) -> p c f", c=2), t_r2[c0 // 2]
                    )
                    nc.gpsimd.dma_start(
                        nt[:].rearrange("p (c f) -> p c f", c=2), n_r2[c0 // 2]
                    )
                    for cc in range(2):
                        c = c0 + cc
                        sls = [
                            slice(cc * FREE + k * CHUNK, cc * FREE + (k + 1) * CHUNK)
                            for k in range(NCHUNK)
                        ]
                        reduce_part(
                            tt, nt, slice(cc * FREE, (cc + 1) * FREE), FREE,
                            c, C + c, c, sls, False,
                        )
                elif kind == "single":
                    tt = tp.tile([P, FREE], bf16)
                    nt = npool.tile([P, FREE], bf16)
                    nc.gpsimd.dma_start(tt[:], t_rs[c0])
                    nc.gpsimd.dma_start(nt[:], n_rs[c0])
                    sls = [slice(k * CHUNK, (k + 1) * CHUNK) for k in range(NCHUNK)]
                    reduce_part(
                        tt, nt, slice(0, FREE), FREE, c0, C + c0, c0, sls, False
                    )
                else:  # "half": last plane in two half-plane tiles
                    for h in range(2):
                        tt = tp.tile([P, HALF], bf16)
                        nt = npool.tile([P, HALF], bf16)
                        nc.gpsimd.dma_start(tt[:], t_rh[c0, h])
                        nc.gpsimd.dma_start(nt[:], n_rh[c0, h])
                        sls = [
                            slice(k * CHUNK, (k + 1) * CHUNK)
                            for k in range(HALF // CHUNK)
                        ]
                        if h == 0:
                            reduce_part(
                                tt, nt, slice(0, HALF), HALF,
                                c0, C + c0, c0, sls, False,
                            )
                        else:
                            reduce_part(
                                tt, nt, slice(0, HALF), HALF,
                                2 * C, 2 * C + 1, c0, sls, True,
                            )

            sb_t = outp.tile([C, 1], f32)
            nc.vector.reduce_sum(sb_t[:], ps_t[:], axis=mybir.AxisListType.X)
            nc.tensor.matmul(ps_tn[:, :], ones[:], acc[:], start=True, stop=True)
            sb = outp.tile([1, NOUT], f32)
            nc.vector.tensor_copy(sb[:], ps_tn[:])
            # two small outputs on different queues so they overlap
            nc.scalar.dma_start(out_tnm.ap(), sb_t[:])
            nc.sync.dma_start(out_tn.ap(), sb[:])

    nc.compile()
    return nc


def _build_v4(C=C, H=H, W=W, num_devices=NCORES, bufs=5):
    """skip-m, all-ACT variant: DVE does fused mul+rowsum (tn), ACT does
    accumulating rowsums of t and n.  No PE in the main loop at all (one
    final [1, 3C+3] partition-reduce matmul) -> ~130 fewer Tensor
    instructions, which shrinks the end-of-kernel semaphore drain.
    First DMA op is partition-striped so packets start flowing during
    descriptor generation; the last plane is split into half-planes to
    shorten the post-stream drain (the second half's t-rowsum runs on DVE
    so ACT and DVE drain in parallel)."""
    import concourse.bacc as bacc
    import concourse.mybir as mybir
    import concourse.tile as tile

    P = 128
    FREE = H * W // P  # 2048
    HALF = FREE // 2

    f32 = mybir.dt.float32
    bf16 = mybir.dt.bfloat16
    nc = bacc.Bacc(
        "TRN2", target_bir_lowering=False, debug=False, num_devices=num_devices
    )

    t_in = nc.dram_tensor("t_in", [C, H, W], f32, kind="ExternalInput")
    n_in = nc.dram_tensor("n_in", [C, H, W], f32, kind="ExternalInput")
    NOUT = 3 * C + 3  # tn[C] | n[C] | t[C] | tn15b, n15b, t15b
    out_tn = nc.dram_tensor("out_tn", [1, NOUT], f32, kind="ExternalOutput")

    # plane pairs interleaved per partition (8KB descriptors)
    t_r2 = t_in.ap().rearrange("(g c) (p a) w -> g p c (a w)", c=2, p=P)
    n_r2 = n_in.ap().rearrange("(g c) (p a) w -> g p c (a w)", c=2, p=P)
    # single planes: [C, P, FREE]
    t_rs = t_in.ap().rearrange("c (p a) w -> c p (a w)", p=P)
    n_rs = n_in.ap().rearrange("c (p a) w -> c p (a w)", p=P)
    # half planes: [C, 2, P, HALF] (4KB descriptors)
    t_rh = t_in.ap().rearrange("c (h p a) w -> c h p (a w)", p=P, a=2)
    n_rh = n_in.ap().rearrange("c (h p a) w -> c h p (a w)", p=P, a=2)

    # singles at both ends (smaller first op, shorter drain), pairs between
    chunks = [("single", 0), ("single", 1)]
    chunks += [("pair", c0) for c0 in range(2, C - 2, 2)]
    chunks += [("single", C - 2), ("half", C - 1)]

    with tile.TileContext(nc) as tc:
        with (
            tc.tile_pool(name="consts", bufs=1) as consts,
            tc.tile_pool(name="tp", bufs=bufs) as tp,
            tc.tile_pool(name="npool", bufs=bufs) as npool,
            tc.tile_pool(name="sp", bufs=2) as sp,
            tc.tile_pool(name="spa", bufs=2) as spa,
            tc.tile_pool(name="outp", bufs=1) as outp,
            tc.tile_pool(name="psum", bufs=1, space="PSUM") as psum,
        ):
            ones = consts.tile([P, 1], f32)
            nc.vector.memset(ones[:], 1.0)
            acc = consts.tile([P, NOUT], f32)
            ps_fin = psum.tile([1, NOUT], f32)

            def reduce_plane(tt, nt, fsl, flen, j_tn, j_n, j_t, t_on_dve):
                # DVE: fused t*n product + per-partition rowsum
                sc = sp.tile([P, flen], bf16)
                nc.vector.scalar_tensor_tensor(
                    out=sc[:],
                    in0=tt[:, fsl],
                    scalar=1.0,
                    in1=nt[:, fsl],
                    op0=mybir.AluOpType.mult,
                    op1=mybir.AluOpType.mult,
                    accum_out=acc[:, j_tn : j_tn + 1],
                )
                # ACT: accumulating rowsum of n
                scn = spa.tile([P, flen], bf16, name="scn")
                nc.scalar.activation(
                    scn[:],
                    nt[:, fsl],
                    mybir.ActivationFunctionType.Copy,
                    accum_out=acc[:, j_n : j_n + 1],
                )
                if t_on_dve:
                    # drain tail: put t's rowsum on DVE so ACT and DVE
                    # finish the last half-plane in parallel
                    nc.vector.reduce_sum(
                        acc[:, j_t : j_t + 1], tt[:, fsl], axis=mybir.AxisListType.X
                    )
                else:
                    sct = spa.tile([P, flen], bf16, name="sct")
                    nc.scalar.activation(
                        sct[:],
                        tt[:, fsl],
                        mybir.ActivationFunctionType.Copy,
                        accum_out=acc[:, j_t : j_t + 1],
                    )

            for kind, c0 in chunks:
                # SWDGE DMAs cast f32 -> bf16 in flight (target is 0/1 so
                # exact; net_out's sums pick up ~1e-6 rel error).
                if kind == "pair":
                    tt = tp.tile([P, 2 * FREE], bf16)
                    nt = npool.tile([P, 2 * FREE], bf16)
                    nc.gpsimd.dma_start(
                        tt[:].rearrange("p (c f) -> p c f", c=2), t_r2[c0 // 2]
                    )
                    nc.gpsimd.dma_start(
                        nt[:].rearrange("p (c f) -> p c f", c=2), n_r2[c0 // 2]
                    )
                    for cc in range(2):
                        c = c0 + cc
                        reduce_plane(
                            tt, nt, slice(cc * FREE, (cc + 1) * FREE), FREE,
                            c, C + c, 2 * C + c, False,
                        )
                elif kind == "single":
                    tt = tp.tile([P, FREE], bf16)
                    nt = npool.tile([P, FREE], bf16)
                    if c0 == 0:
                        # stripe the very first op over partition blocks so
                        # the first descriptors hit the engines early
                        for q in range(4):
                            psl = slice(32 * q, 32 * (q + 1))
                            nc.gpsimd.dma_start(tt[psl, :], t_rs[0, psl])
                    else:
                        nc.gpsimd.dma_start(tt[:], t_rs[c0])
                    nc.gpsimd.dma_start(nt[:], n_rs[c0])
                    reduce_plane(
                        tt, nt, slice(0, FREE), FREE, c0, C + c0, 2 * C + c0, False
                    )
                else:  # "half": last plane in two half-plane tiles
                    for h in range(2):
                        tt = tp.tile([P, HALF], bf16)
                        nt = npool.tile([P, HALF], bf16)
                        nc.gpsimd.dma_start(tt[:], t_rh[c0, h])
                        nc.gpsimd.dma_start(nt[:], n_rh[c0, h])
                        if h == 0:
                            reduce_plane(
                                tt, nt, slice(0, HALF), HALF,
                                c0, C + c0, 2 * C + c0, False,
                            )
                        else:
                            reduce_plane(
                                tt, nt, slice(0, HALF), HALF,
                                3 * C, 3 * C + 1, 3 * C + 2, True,
                            )

            # partition-axis total: [128, NOUT] -> [1, NOUT]
            nc.tensor.matmul(ps_fin[:, :], ones[:], acc[:], start=True, stop=True)
            sb = outp.tile([1, NOUT], f32)
            nc.vector.tensor_copy(sb[:], ps_fin[:])
            nc.sync.dma_start(out_tn.ap(), sb[:])

    nc.compile()
    return nc


def _build_f32(C=C, H=H, W=W, num_devices=NCORES, bufs=3):
    """All-f32 variant: HWDGE loads (no cast), no TensorE in the main loop.
    DVE: fused mul+rowsum of t*n, plus rowsum of t.  ACT: accumulating
    rowsums of n and m.  One final f32 matmul reduces the [128, 4C]
    accumulator across partitions."""
    import concourse.bacc as bacc
    import concourse.mybir as mybir
    import concourse.tile as tile

    P = 128
    FREE = H * W // P

    f32 = mybir.dt.float32
    nc = bacc.Bacc(
        "TRN2", target_bir_lowering=False, debug=False, num_devices=num_devices
    )

    t_in = nc.dram_tensor("t_in", [C, H, W], f32, kind="ExternalInput")
    n_in = nc.dram_tensor("n_in", [C, H, W], f32, kind="ExternalInput")
    m_in = nc.dram_tensor("m_in", [C, H, W], f32, kind="ExternalInput")
    out_tn = nc.dram_tensor("out_tn", [1, 4 * C], f32, kind="ExternalOutput")

    t_r = t_in.ap().rearrange("c (p a) w -> c p (a w)", p=P)
    n_r = n_in.ap().rearrange("c (p a) w -> c p (a w)", p=P)
    m_r = m_in.ap().rearrange("c (p a) w -> c p (a w)", p=P)

    with tile.TileContext(nc) as tc:
        with (
            tc.tile_pool(name="consts", bufs=1) as consts,
            tc.tile_pool(name="tp", bufs=bufs) as tp,
            tc.tile_pool(name="npool", bufs=bufs) as npool,
            tc.tile_pool(name="mp", bufs=bufs) as mp,
            tc.tile_pool(name="sp", bufs=2) as sp,
            tc.tile_pool(name="spa", bufs=2) as spa,
            tc.tile_pool(name="outp", bufs=1) as outp,
            tc.tile_pool(name="psum", bufs=1, space="PSUM") as psum,
        ):
            ones = consts.tile([P, 1], f32)
            nc.vector.memset(ones[:], 1.0)
            # cols [0,C)=t*n  [C,2C)=m  [2C,3C)=t  [3C,4C)=n
            acc = consts.tile([P, 4 * C], f32)
            ps_fin = psum.tile([1, 4 * C], f32)

            for c in range(C):
                tt = tp.tile([P, FREE], f32, name="tt")
                nc.sync.dma_start(tt[:], t_r[c])
                nt = npool.tile([P, FREE], f32, name="nt")
                nc.scalar.dma_start(nt[:], n_r[c])
                mt = mp.tile([P, FREE], f32, name="mt")
                nc.sync.dma_start(mt[:], m_r[c])

                sc = sp.tile([P, FREE], f32, name="sc")
                nc.vector.scalar_tensor_tensor(
                    out=sc[:],
                    in0=tt[:],
                    scalar=1.0,
                    in1=nt[:],
                    op0=mybir.AluOpType.mult,
                    op1=mybir.AluOpType.mult,
                    accum_out=acc[:, c : c + 1],
                )
                nc.vector.reduce_sum(
                    acc[:, 2 * C + c : 2 * C + c + 1],
                    tt[:],
                    axis=mybir.AxisListType.X,
                )
                scn = spa.tile([P, FREE], f32, name="scn")
                nc.scalar.activation(
                    scn[:],
                    nt[:],
                    mybir.ActivationFunctionType.Copy,
                    accum_out=acc[:, 3 * C + c : 3 * C + c + 1],
                )
                scm = spa.tile([P, FREE], f32, name="scm")
                nc.scalar.activation(
                    scm[:],
                    mt[:],
                    mybir.ActivationFunctionType.Copy,
                    accum_out=acc[:, C + c : C + c + 1],
                )

            nc.tensor.matmul(ps_fin[:, :], ones[:], acc[:], start=True, stop=True)
            sb = outp.tile([1, 4 * C], f32)
            nc.vector.tensor_copy(sb[:], ps_fin[:])
            nc.sync.dma_start(out_tn.ap(), sb[:])

    nc.compile()
    return nc


_V = os.environ.get("K_V", "7")
_M_ON = os.environ.get("K_M_ON", "skip")
_N_ON = os.environ.get("K_N_ON", "act")
_BUFS = int(os.environ.get("K_BUFS", "5"))
_CPT = int(os.environ.get("K_CPT", "2"))
_EDGE = os.environ.get("K_EDGE", "1") == "1"


def _get_nc():
    if "nc" not in _CACHE:
        if _V == "7":
            _CACHE["nc"] = _build_v7(bufs=min(_BUFS, 4))
        elif _V == "5":
            _CACHE["nc"] = _build_v5(bufs=_BUFS)
        elif _V == "4":
            _CACHE["nc"] = _build_v4(bufs=_BUFS)
        else:
            _CACHE["nc"] = _build(
                m_on=_M_ON,
                n_on=_N_ON,
                bufs=_BUFS,
                cpt=_CPT,
                edge_split=_EDGE,
                m_f32=os.environ.get("K_MF32", "0") == "1",
            )
    return _CACHE["nc"]


def _run(net_out, target, max_positiones, trace=False):
    from concourse.bass_utils import run_bass_kernel_spmd

    nc = _get_nc()
    in_maps = []
    for i in range(NCORES):
        im = {
            "t_in": np.ascontiguousarray(target[i]),
            "n_in": np.ascontiguousarray(net_out[i]),
        }
        if (_V not in ("4", "5", "7") and _M_ON != "skip") or (
            _V == "5" and os.environ.get("K_PROBE_HW", "0") != "0"
        ):
            im["m_in"] = np.ascontiguousarray(max_positiones[i])
        in_maps.append(im)
    res = run_bass_kernel_spmd(
        nc, in_maps, core_ids=list(range(NCORES)), trace=trace
    )
    return res


def _finish(results, max_positiones=None, n_in_tn=False):
    # results: list (per core) of {"out_tn": [1,2C] (tn | m-or-n sums),
    #                               "out_tnm": [C,3] (t, n, m; unused cols 0)}
    if results[0]["out_tn"].shape[0] == 3:  # v7: [3, 23] (all|upper|lower)
        full = np.stack([r["out_tn"] for r in results]).astype(np.float64)
        B_ = len(results)
        tn = np.empty((B_, 16))
        sn = np.empty((B_, 16))
        tn[:, 0], sn[:, 0] = full[:, 0, 20], full[:, 0, 21]
        tn[:, 1], sn[:, 1] = full[:, 0, 12], full[:, 0, 13]
        for c in range(2, 14):
            p = (c - 2) // 2
            row = 1 if c % 2 == 0 else 2  # even plane = upper partitions
            tn[:, c], sn[:, c] = full[:, row, p], full[:, row, 6 + p]
        tn[:, 14], sn[:, 14] = full[:, 0, 14], full[:, 0, 15]
        tn[:, 15] = full[:, 0, 16] + full[:, 0, 18]
        sn[:, 15] = full[:, 0, 17] + full[:, 0, 19]
        st = np.stack([r["out_tnm"][:, 0] for r in results]).astype(np.float64)
        st[:, 0] = full[:, 0, 22]
        sm = np.zeros_like(st)
        return _loss_tail(tn, sn, st, sm, max_positiones)
    tnm_flat = np.stack([r["out_tn"][0] for r in results]).astype(np.float64)
    if tnm_flat.shape[1] in (34, 35):  # v5/v6: tn[C] | n[C] | tn15b, n15b[, t0]
        tn = tnm_flat[:, 0:16].copy()
        sn = tnm_flat[:, 16:32].copy()
        tn[:, 15] += tnm_flat[:, 32]
        sn[:, 15] += tnm_flat[:, 33]
        st = np.stack([r["out_tnm"][:, 0] for r in results]).astype(np.float64)
        if tnm_flat.shape[1] == 35:  # head plane's t summed on ACT, not PE
            st[:, 0] = tnm_flat[:, 34]
        sm = np.zeros_like(st)
    elif tnm_flat.shape[1] == 51:  # v4: tn[C] | n[C] | t[C] | tn15b,n15b,t15b
        tn = tnm_flat[:, 0:16].copy()
        sn = tnm_flat[:, 16:32].copy()
        st = tnm_flat[:, 32:48].copy()
        tn[:, 15] += tnm_flat[:, 48]
        sn[:, 15] += tnm_flat[:, 49]
        st[:, 15] += tnm_flat[:, 50]
        sm = np.zeros_like(st)
    elif tnm_flat.shape[1] == 64:  # all-f32 layout: tn | m | t | n
        tn, sm = tnm_flat[:, :16], tnm_flat[:, 16:32]
        st, sn = tnm_flat[:, 32:48], tnm_flat[:, 48:64]
    elif n_in_tn:  # skip mode with n on ACT: out_tn = tn | n, out_tnm col0 = t
        tn, sn = tnm_flat[:, :16], tnm_flat[:, 16:]
        tnm = np.stack([r["out_tnm"] for r in results]).astype(np.float64)
        st = tnm[..., 0]
        sm = np.zeros_like(st)
    else:
        tn, sm_a = tnm_flat[:, :16], tnm_flat[:, 16:]  # [B,C] each
        tnm = np.stack([r["out_tnm"] for r in results]).astype(np.float64)
        st, sn, sm_b = tnm[..., 0], tnm[..., 1], tnm[..., 2]
        sm = sm_a + sm_b  # exactly one of the two paths populated its slot

    return _loss_tail(tn, sn, st, sm, max_positiones)


def _loss_tail(tn, sn, st, sm, max_positiones=None):
    b2 = 1.5 * 1.5
    w1 = b2 / (1.0 + b2)
    w2 = 1.0 / (1.0 + b2)
    molecule = tn
    fn = st - tn
    fp = sn - tn
    loss = 1.0 - molecule / (molecule + w1 * fn + w2 * fp)
    active = (st > 0) | (sm > 0)
    if max_positiones is not None:
        # device skipped max_positiones (it only matters for planes whose
        # target is entirely zero); resolve those few planes on host.
        for b, c in zip(*np.nonzero(~active)):
            if np.max(max_positiones[b, c]) > 0:
                active[b, c] = True
    losses = np.where(active, loss, 0.0)
    cnt = np.sum(losses != 0, axis=1).astype(np.float64)
    img_losses = np.sum(losses, axis=1) / cnt
    out = np.sum(img_losses) / img_losses.shape[0]
    return np.asarray(out, dtype=np.float32)


def kernel(net_out, target, max_positiones):
    net_out = np.asarray(net_out, dtype=np.float32)
    target = np.asarray(target, dtype=np.float32)
    max_positiones = np.asarray(max_positiones, dtype=np.float32)
    res = _run(net_out, target, max_positiones, trace=False)
    return _finish(
        res.results,
        max_positiones if (_M_ON == "skip" or _V in ("4", "5", "7")) else None,
        n_in_tn=(_N_ON == "act" and _M_ON == "skip"),
    )

